# revision 1
# baseline (speedup 1.0000x reference)
"""LinearSelfAttention kernel for TRN2 (8 NeuronCores, batch-parallel).

Computes out = H + (PH @ mask(H^T Q H)) / n per sample, re-associated as
    HtQ = H^T Q            [s, e]
    PHt = (P H)^T          [s, d]
    Ct  = HtQ[:n]^T PHt[:n]  [e, d]   (mask = drop s == n row)
    out = H + (Ct/n)^T H
which is O(n d^2) instead of O(n^2 d).

Sharding: data-parallel over batch, 2 samples per core, P/Q replicated.
Matmuls in bf16 (fp32 PSUM accumulate); the fp32 H is added in the
epilogue on DVE so the dominant H term stays exact.
"""

import sys

sys.path.insert(0, "/opt/trn_rl_repo")

import numpy as np
import ml_dtypes

B, D1, N1 = 16, 257, 2049  # batch, d+1, n+1
N = N1 - 1  # 2048
NCORES = 8
BPC = B // NCORES  # samples per core

# partition chunking of the 257-sized dims: (offset, size)
CH = [(0, 128), (128, 128), (256, 1)]
NT = N // 128  # 16 full s-tiles (s == 2048 row is masked off)
# t chunks for the final matmul free dim
TCH = [(i * 512, min(512, N1 - i * 512)) for i in range((N1 + 511) // 512)]

_cached = {}


def _build():
    import concourse.bass as bass
    import concourse.tile as tile
    from concourse import bacc, mybir
    from contextlib import ExitStack

    f32 = mybir.dt.float32
    bf16 = mybir.dt.bfloat16

    nc = bacc.Bacc("TRN2", target_bir_lowering=False, debug=False, num_devices=NCORES)

    H_d = nc.declare_dram_parameter("H", [BPC, D1, N1], f32, isOutput=False)
    Hb_d = nc.declare_dram_parameter("Hb", [BPC, D1, N1], bf16, isOutput=False)
    QP_d = nc.declare_dram_parameter("QP", [D1, 514], bf16, isOutput=False)
    Y_d = nc.declare_dram_parameter("Y", [BPC, D1, N1], f32, isOutput=True)

    with tile.TileContext(nc) as tc:
        with ExitStack() as ctx:
            const = ctx.enter_context(tc.tile_pool(name="const", bufs=1))
            hfp = ctx.enter_context(tc.tile_pool(name="hfp", bufs=2))
            hbp = ctx.enter_context(tc.tile_pool(name="hbp", bufs=2))
            sq = ctx.enter_context(tc.tile_pool(name="sq", bufs=2))
            ctp = ctx.enter_context(tc.tile_pool(name="ctp", bufs=2))
            yp = ctx.enter_context(tc.tile_pool(name="yp", bufs=6))

            # ---- input DMAs, spread across engine queues so the first
            # s-tile's operands land as early as possible
            qp = []
            for c, (off, sz) in enumerate(CH):
                t = const.tile([128, 514], bf16, tag=f"qp{c}", name=f"qp{c}")
                nc.sync.dma_start(t[:sz, :], QP_d[off : off + sz, :])
                qp.append(t)

            hf = [[None] * 3 for _ in range(BPC)]
            hb = [[None] * 3 for _ in range(BPC)]
            # Priority class 0: sample-0 bf16 H (gates the first matmuls).
            # One tile per queue so it gets the full HBM bandwidth.
            load_eng = {0: nc.sync, 1: nc.scalar, 2: nc.gpsimd}
            for c, (off, sz) in enumerate(CH):
                tb = hbp.tile([128, N1], bf16, tag=f"hb{c}", name=f"hb0_{c}")
                load_eng[c].dma_start(tb[:sz, :], Hb_d[0, off : off + sz, :])
                hb[0][c] = tb
            # Priority class 1+2 (gpsimd, gated behind class 0 by probe
            # copies): sample-1 bf16 H, then the fp32 H for the epilogue.
            probe = const.tile([128, 16], bf16, tag="probe", name="probe")
            nc.gpsimd.tensor_copy(probe[0:1, 0:8], hb[0][0][0:1, 0:8])
            nc.gpsimd.tensor_copy(probe[0:1, 8:16], hb[0][1][0:1, 0:8])
            for c, (off, sz) in enumerate(CH):
                tb = hbp.tile([128, N1], bf16, tag=f"hb{c}", name=f"hb1_{c}")
                nc.gpsimd.dma_start(tb[:sz, :], Hb_d[1, off : off + sz, :])
                hb[1][c] = tb
            for b in range(BPC):
                for c, (off, sz) in enumerate(CH):
                    tf = hfp.tile([128, N1], f32, tag=f"hf{c}", name=f"hf{b}_{c}")
                    nc.gpsimd.dma_start(tf[:sz, :], H_d[b, off : off + sz, :])
                    hf[b][c] = tf

            # ---- PE warmup: dummy matmuls bridge the input-DMA latency and
            # push the HAM clock gate to K=8/8. Results never read.
            wsb = const.tile([128, 128], bf16, tag="wsb", name="wsb")
            nc.vector.memset(wsb[:, :], 0.0)
            with tc.tile_pool(name="wp", bufs=1, space="PSUM") as wp:
                wps = wp.tile([128, 512], f32, tag="wps", name="warm_ps")
                NWARM = 145
                for i in range(NWARM):
                    nc.tensor.matmul(
                        wps[:, 0:128],
                        wsb[:, :],
                        wsb[:, :],
                        start=(i == 0),
                        stop=(i == NWARM - 1),
                    )

            htq = [None] * BPC
            pht = [None] * BPC
            # ---- S1 + S2: HtQ [s,e] and PHt [s,d], 16 s-tiles each ----
            with tc.tile_pool(name="pp12", bufs=3, space="PSUM") as pp12:
                for b in range(BPC):
                    htq[b] = sq.tile([128, NT * 257], bf16, tag="htq", name=f"htq{b}")
                    pht[b] = sq.tile([128, NT * 257], bf16, tag="pht", name=f"pht{b}")
                    for st in range(NT):
                        p_htq = pp12.tile(
                            [128, 257], f32, tag="p_htq", name=f"p_htq{b}_{st}"
                        )
                        p_pht = pp12.tile(
                            [128, 257], f32, tag="p_pht", name=f"p_pht{b}_{st}"
                        )
                        sl = slice(st * 128, (st + 1) * 128)
                        for c, (off, sz) in enumerate(CH):
                            st_flags = dict(start=(c == 0), stop=(c == 2))
                            nc.tensor.matmul(
                                p_htq[:, :],
                                hb[b][c][:sz, sl],
                                qp[c][:sz, 0:257],
                                **st_flags,
                            )
                            nc.tensor.matmul(
                                p_pht[:, :],
                                hb[b][c][:sz, sl],
                                qp[c][:sz, 257:514],
                                **st_flags,
                            )
                        osl = slice(st * 257, (st + 1) * 257)
                        # alternate eviction engines to balance DVE/ACT
                        if st % 2 == 0:
                            nc.vector.tensor_copy(htq[b][:, osl], p_htq[:, :])
                            nc.scalar.copy(pht[b][:, osl], p_pht[:, :])
                        else:
                            nc.scalar.copy(htq[b][:, osl], p_htq[:, :])
                            nc.vector.tensor_copy(pht[b][:, osl], p_pht[:, :])

            # ---- S3: Ct[e,d] = sum_{s<2048} HtQ[s,e] * PHt[s,d], scaled 1/n
            ct = [[None] * 3 for _ in range(BPC)]
            with tc.tile_pool(name="pp3", bufs=3, space="PSUM") as pp3:
                for b in range(BPC):
                    for ec, (eoff, esz) in enumerate(CH):
                        p_ct = pp3.tile([128, 257], f32, tag="p_ct", name=f"p_ct{b}_{ec}")
                        for st in range(NT):
                            base = st * 257
                            nc.tensor.matmul(
                                p_ct[:esz, :],
                                htq[b][:, base + eoff : base + eoff + esz],
                                pht[b][:, base : base + 257],
                                start=(st == 0),
                                stop=(st == NT - 1),
                            )
                        t = ctp.tile([128, 257], bf16, tag=f"ct{ec}", name=f"ct{b}_{ec}")
                        nc.scalar.mul(t[:esz, :], p_ct[:esz, :], 1.0 / N)
                        ct[b][ec] = t

            # ---- S4: Y = H + (Ct/n)^T H ----
            with tc.tile_pool(name="pp4", bufs=4, space="PSUM") as pp4:
                for b in range(BPC):
                    for dc, (doff, dsz) in enumerate(CH):
                        y = yp.tile([128, N1], f32, tag="y", name=f"y{b}_{dc}")
                        for toff, tsz in TCH:
                            p_a = pp4.tile(
                                [128, 512], f32, tag="p_a", name=f"p_a{b}_{dc}_{toff}"
                            )
                            for ec, (eoff, esz) in enumerate(CH):
                                nc.tensor.matmul(
                                    p_a[:dsz, :tsz],
                                    ct[b][ec][:esz, doff : doff + dsz],
                                    hb[b][ec][:esz, toff : toff + tsz],
                                    start=(ec == 0),
                                    stop=(ec == 2),
                                )
                            nc.vector.tensor_add(
                                y[:dsz, toff : toff + tsz],
                                p_a[:dsz, :tsz],
                                hf[b][dc][:dsz, toff : toff + tsz],
                            )
                            # store each chunk as soon as its epilogue add is
                            # done; alternate queues so store issue keeps up
                            st_eng = nc.sync if (toff // 512) % 2 == 0 else nc.scalar
                            st_eng.dma_start(
                                Y_d[b, doff : doff + dsz, toff : toff + tsz],
                                y[:dsz, toff : toff + tsz],
                            )

    nc.compile()
    return nc


def _prep_in_maps(H, P, Q):
    H = np.ascontiguousarray(H, dtype=np.float32)
    Hb = H.astype(ml_dtypes.bfloat16)
    QP = np.ascontiguousarray(
        np.concatenate([Q, P.T], axis=1).astype(ml_dtypes.bfloat16)
    )
    return [
        {
            "H": H[c * BPC : (c + 1) * BPC],
            "Hb": Hb[c * BPC : (c + 1) * BPC],
            "QP": QP,
        }
        for c in range(NCORES)
    ]


def kernel(H, P, Q):
    from concourse.bass_utils import run_bass_kernel_spmd

    if "nc" not in _cached:
        _cached["nc"] = _build()
    nc = _cached["nc"]

    in_maps = _prep_in_maps(H, P, Q)
    res = run_bass_kernel_spmd(nc, in_maps, list(range(NCORES)))
    out = np.concatenate([res.results[c]["Y"] for c in range(NCORES)], axis=0)
    return out.astype(np.float32)



# revision 2
# speedup vs baseline: 1.1418x; 1.1418x over previous
"""LinearSelfAttention kernel for TRN2 (8 NeuronCores, batch-parallel).

Key identity: with Hn = H[:, :n] (mask drops column n from the s-sum),
    attn = P H mask(H^T Q H) = C H,   C = P G Q,   G = Hn Hn^T  (257x257)
so  out = H + C H / n = (I + C/n) H = Et^T H,  Et = I + Q^T G P^T / n.
This is O(n d^2) for G and Et^T H plus O(d^3) for the tiny chain,
vs O(3 n d^2) for the naive re-association -- and the +H epilogue is
folded into the matmul via the identity.

Sharding: data-parallel over batch, 2 samples per core. Host ships
H in bf16 twice (as-is for the Et^T H stream, transposed for G) plus
a packed const [Q/n | P^T | I]. Output returned in bf16, cast on host.
"""

import sys

sys.path.insert(0, "/opt/trn_rl_repo")

import numpy as np
import ml_dtypes

B, D1, N1 = 16, 257, 2049  # batch, d+1, n+1
N = N1 - 1  # 2048
NCORES = 8
BPC = B // NCORES  # samples per core

# partition chunking of the 257-sized dims: (offset, size)
CH = [(0, 128), (128, 128), (256, 1)]
NT = N // 128  # 16 s-tiles of the transposed Hn
# t chunks for the final matmul free dim (PSUM bank = 512 fp32)
TCH = [(i * 512, min(512, N1 - i * 512)) for i in range((N1 + 511) // 512)]
NWARM = 60

_cached = {}


def _build():
    import concourse.bass as bass
    import concourse.tile as tile
    from concourse import bacc, mybir
    from contextlib import ExitStack

    f32 = mybir.dt.float32
    bf16 = mybir.dt.bfloat16

    nc = bacc.Bacc("TRN2", target_bir_lowering=False, debug=False, num_devices=NCORES)

    Hb_d = nc.declare_dram_parameter("Hb", [BPC, D1, N1], bf16, isOutput=False)
    Ht_d = nc.declare_dram_parameter("Ht", [BPC, N, D1], bf16, isOutput=False)
    QPI_d = nc.declare_dram_parameter("QPI", [D1, 3 * D1], bf16, isOutput=False)
    Y_d = nc.declare_dram_parameter("Y", [BPC, D1, N1], bf16, isOutput=True)

    with tile.TileContext(nc) as tc:
        with ExitStack() as ctx:
            const = ctx.enter_context(tc.tile_pool(name="const", bufs=1))
            htp = ctx.enter_context(tc.tile_pool(name="htp", bufs=2))
            hbp = ctx.enter_context(tc.tile_pool(name="hbp", bufs=2))
            sq = ctx.enter_context(tc.tile_pool(name="sq", bufs=2))
            yp = ctx.enter_context(tc.tile_pool(name="yp", bufs=2))

            # ---- input DMAs round-robined over three queues; per-queue FIFO
            # keeps sample 0 ahead of sample 1
            qs = [nc.sync, nc.scalar, nc.gpsimd]
            qi = 0

            def load(dst, src):
                nonlocal qi
                qs[qi % 3].dma_start(dst, src)
                qi += 1

            qpi = []
            for c, (off, sz) in enumerate(CH):
                t = const.tile([128, 3 * D1], bf16, tag=f"qpi{c}", name=f"qpi{c}")
                load(t[:sz, :], QPI_d[off : off + sz, :])
                qpi.append(t)

            ht = [[None] * NT for _ in range(BPC)]
            hb = [[None] * 3 for _ in range(BPC)]
            for b in range(BPC):
                for st in range(NT):
                    t = htp.tile([128, D1], bf16, tag=f"ht{st}", name=f"ht{b}_{st}")
                    load(t[:, :], Ht_d[b, st * 128 : (st + 1) * 128, :])
                    ht[b][st] = t
                for c, (off, sz) in enumerate(CH):
                    t = hbp.tile([128, N1], bf16, tag=f"hb{c}", name=f"hb{b}_{c}")
                    load(t[:sz, :], Hb_d[b, off : off + sz, :])
                    hb[b][c] = t

            # ---- PE warmup: bridge input-DMA latency, ramp the HAM clock
            wsb = const.tile([128, 128], bf16, tag="wsb", name="wsb")
            nc.vector.memset(wsb[:, :], 0.0)
            with tc.tile_pool(name="wp", bufs=1, space="PSUM") as wp:
                wps = wp.tile([128, 512], f32, tag="wps", name="warm_ps")
                for i in range(NWARM):
                    nc.tensor.matmul(
                        wps[:, 0:128],
                        wsb[:, :],
                        wsb[:, :],
                        start=(i == 0),
                        stop=(i == NWARM - 1),
                    )

            with (
                tc.tile_pool(name="ppg", bufs=1, space="PSUM") as ppg,
                tc.tile_pool(name="pp4", bufs=4, space="PSUM") as pp4,
            ):
                for b in range(BPC):
                    # ---- G = Hn Hn^T, 3 partition chunks of the output rows
                    g_ps = [
                        ppg.tile([128, D1], f32, tag=f"g{ac}", name=f"g_ps{b}_{ac}")
                        for ac in range(3)
                    ]
                    for st in range(NT):
                        for ac, (aoff, asz) in enumerate(CH):
                            nc.tensor.matmul(
                                g_ps[ac][:asz, :],
                                ht[b][st][:, aoff : aoff + asz],
                                ht[b][st][:, :],
                                start=(st == 0),
                                stop=(st == NT - 1),
                            )
                    gsb = []
                    for ac, (aoff, asz) in enumerate(CH):
                        t = sq.tile([128, D1], bf16, tag=f"g{ac}", name=f"gs{b}_{ac}")
                        eng = nc.scalar.copy if ac % 2 == 0 else nc.vector.tensor_copy
                        eng(t[:asz, :], g_ps[ac][:asz, :])
                        gsb.append(t)

                    # ---- V = G P^T  (G symmetric: lhsT slices G directly)
                    v_ps = [
                        ppg.tile([128, D1], f32, tag=f"g{am}", name=f"v_ps{b}_{am}")
                        for am in range(3)
                    ]
                    for am, (amoff, amsz) in enumerate(CH):
                        for kb, (kboff, kbsz) in enumerate(CH):
                            nc.tensor.matmul(
                                v_ps[am][:amsz, :],
                                gsb[kb][:kbsz, amoff : amoff + amsz],
                                qpi[kb][:kbsz, D1 : 2 * D1],
                                start=(kb == 0),
                                stop=(kb == 2),
                            )
                    vsb = []
                    for am, (amoff, amsz) in enumerate(CH):
                        t = sq.tile([128, D1], bf16, tag=f"v{am}", name=f"vs{b}_{am}")
                        eng = nc.scalar.copy if am % 2 == 1 else nc.vector.tensor_copy
                        eng(t[:amsz, :], v_ps[am][:amsz, :])
                        vsb.append(t)

                    # ---- Et = I + (Q/n)^T V  (= I + C^T/n)
                    c_ps = [
                        ppg.tile([128, D1], f32, tag=f"g{em}", name=f"c_ps{b}_{em}")
                        for em in range(3)
                    ]
                    for em, (emoff, emsz) in enumerate(CH):
                        for ka, (kaoff, kasz) in enumerate(CH):
                            nc.tensor.matmul(
                                c_ps[em][:emsz, :],
                                qpi[ka][:kasz, emoff : emoff + emsz],
                                vsb[ka][:kasz, :],
                                start=(ka == 0),
                                stop=(ka == 2),
                            )
                    et = []
                    for em, (emoff, emsz) in enumerate(CH):
                        t = sq.tile([128, D1], bf16, tag=f"e{em}", name=f"et{b}_{em}")
                        nc.vector.tensor_add(
                            t[:emsz, :],
                            c_ps[em][:emsz, :],
                            qpi[em][:emsz, 2 * D1 : 3 * D1],
                        )
                        et.append(t)

                    # ---- Y = Et^T H
                    ei = 0
                    for dc, (doff, dsz) in enumerate(CH):
                        y = yp.tile([128, N1], bf16, tag=f"y{dc}", name=f"y{b}_{dc}")
                        for toff, tsz in TCH:
                            p = pp4.tile(
                                [128, 512], f32, tag="p", name=f"p4_{b}_{dc}_{toff}"
                            )
                            for ec, (eoff, esz) in enumerate(CH):
                                nc.tensor.matmul(
                                    p[:dsz, :tsz],
                                    et[ec][:esz, doff : doff + dsz],
                                    hb[b][ec][:esz, toff : toff + tsz],
                                    start=(ec == 0),
                                    stop=(ec == 2),
                                )
                            eng = (
                                nc.scalar.copy if ei % 2 == 0 else nc.vector.tensor_copy
                            )
                            eng(y[:dsz, toff : toff + tsz], p[:dsz, :tsz])
                            ei += 1
                        st_eng = nc.sync if dc % 2 == 0 else nc.gpsimd
                        st_eng.dma_start(Y_d[b, doff : doff + dsz, :], y[:dsz, :])

    nc.compile()
    return nc


def _prep_in_maps(H, P, Q):
    bf = ml_dtypes.bfloat16
    H = np.ascontiguousarray(H, dtype=np.float32)
    Hb = H.astype(bf)
    Ht = np.ascontiguousarray(np.swapaxes(H[:, :, :N], 1, 2)).astype(bf)
    QPI = np.concatenate(
        [Q / N, P.T, np.eye(D1, dtype=np.float32)], axis=1
    ).astype(bf)
    QPI = np.ascontiguousarray(QPI)
    return [
        {
            "Hb": Hb[c * BPC : (c + 1) * BPC],
            "Ht": Ht[c * BPC : (c + 1) * BPC],
            "QPI": QPI,
        }
        for c in range(NCORES)
    ]


def kernel(H, P, Q):
    from concourse.bass_utils import run_bass_kernel_spmd

    if "nc" not in _cached:
        _cached["nc"] = _build()
    nc = _cached["nc"]

    in_maps = _prep_in_maps(H, P, Q)
    res = run_bass_kernel_spmd(nc, in_maps, list(range(NCORES)))
    out = np.concatenate([res.results[c]["Y"] for c in range(NCORES)], axis=0)
    return out.astype(np.float32)


# revision 10
# speedup vs baseline: 1.5798x; 1.3836x over previous
"""LinearSelfAttention kernel for TRN2 (8 NeuronCores, batch-parallel).

Key identity: with Hn = H[:, :n] (mask drops column n from the s-sum),
    attn = P H mask(H^T Q H) = C H,   C = P G Q,   G = Hn Hn^T  (257x257)
so  out = H + C H / n = (I + C/n) H = Et^T H,  Et = I + Q^T G P^T / n.
This is O(n d^2) for G and Et^T H plus O(d^3) for the tiny chain,
vs O(3 n d^2) for the naive re-association -- and the +H epilogue is
folded into the matmul via the identity.

Sharding: data-parallel over batch, 2 samples per core. Host ships
H in bf16 twice (as-is for the Et^T H stream, transposed for G) plus
a packed const [Q/n | P^T | I]. Output returned in bf16, cast on host.
"""

import sys

sys.path.insert(0, "/opt/trn_rl_repo")

import numpy as np
import ml_dtypes

B, D1, N1 = 16, 257, 2049  # batch, d+1, n+1
N = N1 - 1  # 2048
NCORES = 8
BPC = B // NCORES  # samples per core

# partition chunking of the 257-sized dims: (offset, size)
CH = [(0, 128), (128, 128), (256, 1)]
NT8 = N // 256  # 8 double-row s-tiles of the transposed Hn (fp8 DoubleRow)
DPAD = 272  # fp8 DR LDWEIGHTS: step between the 2 K-subtiles must be %16==0
# t chunks for the final matmul free dim (PSUM bank = 512 fp32)
TCH = [(i * 512, min(512, N1 - i * 512)) for i in range((N1 + 511) // 512)]
NWARM = 30

_cached = {}


def _build():
    import concourse.bass as bass
    import concourse.tile as tile
    from concourse import bacc, mybir
    from contextlib import ExitStack

    f32 = mybir.dt.float32
    bf16 = mybir.dt.bfloat16
    f8 = mybir.dt.float8e4
    DR = mybir.MatmulPerfMode.DoubleRow

    nc = bacc.Bacc("TRN2", target_bir_lowering=False, debug=False, num_devices=NCORES)

    Hb_d = nc.declare_dram_parameter("Hb", [BPC, D1, N1], bf16, isOutput=False)
    Ht_d = nc.declare_dram_parameter("Ht", [BPC, NT8, 128, 2, DPAD], f8, isOutput=False)
    QPI_d = nc.declare_dram_parameter("QPI", [D1, 3 * D1], bf16, isOutput=False)
    Y_d = nc.declare_dram_parameter("Y", [BPC, D1, N1], bf16, isOutput=True)

    with tile.TileContext(nc) as tc:
        with ExitStack() as ctx:
            const = ctx.enter_context(tc.tile_pool(name="const", bufs=1))
            htp = ctx.enter_context(tc.tile_pool(name="htp", bufs=2))
            hbp = ctx.enter_context(tc.tile_pool(name="hbp", bufs=2))
            sq = ctx.enter_context(tc.tile_pool(name="sq", bufs=2))
            yp = ctx.enter_context(tc.tile_pool(name="yp", bufs=2))

            # ---- input DMAs round-robined over three queues; per-queue FIFO
            # keeps sample 0 ahead of sample 1
            qs = [nc.sync, nc.scalar, nc.gpsimd]
            qi = 0

            def load(dst, src):
                nonlocal qi
                qs[qi % 3].dma_start(dst, src)
                qi += 1

            qpi = []
            for c, (off, sz) in enumerate(CH):
                t = const.tile([128, 3 * D1], bf16, tag=f"qpi{c}", name=f"qpi{c}")
                load(t[:sz, :], QPI_d[off : off + sz, :])
                qpi.append(t)

            ht = [[None] * NT8 for _ in range(BPC)]
            hb = [[None] * 3 for _ in range(BPC)]
            for b in range(BPC):
                for st in range(NT8):
                    t = htp.tile([128, 2, DPAD], f8, tag=f"ht{st}", name=f"ht{b}_{st}")
                    load(t[:, :, :], Ht_d[b, st])
                    ht[b][st] = t
                for c, (off, sz) in enumerate(CH):
                    t = hbp.tile([128, N1], bf16, tag=f"hb{c}", name=f"hb{b}_{c}")
                    load(t[:sz, :], Hb_d[b, off : off + sz, :])
                    hb[b][c] = t

            # ---- PE warmup: bridge input-DMA latency, ramp the HAM clock
            wsb = const.tile([128, 128], bf16, tag="wsb", name="wsb")
            nc.vector.memset(wsb[:, :], 0.0)
            with tc.tile_pool(name="wp", bufs=1, space="PSUM") as wp:
                wps = wp.tile([128, 512], f32, tag="wps", name="warm_ps")
                for i in range(NWARM):
                    nc.tensor.matmul(
                        wps[:, 0:128],
                        wsb[:, :],
                        wsb[:, :],
                        start=(i == 0),
                        stop=(i == NWARM - 1),
                    )

            with (
                tc.tile_pool(name="ppg", bufs=1, space="PSUM") as ppg,
                tc.tile_pool(name="pp4", bufs=4, space="PSUM") as pp4,
            ):
                for b in range(BPC):
                    # ---- G = Hn Hn^T, 3 partition chunks of the output rows
                    # (fp8 DoubleRow: K=256 per pass, 0.5 cycles/row)
                    g_ps = [
                        ppg.tile([128, D1], f32, tag=f"g{ac}", name=f"g_ps{b}_{ac}")
                        for ac in range(3)
                    ]
                    for st in range(NT8):
                        for ac, (aoff, asz) in enumerate(CH):
                            nc.tensor.matmul(
                                g_ps[ac][:asz, :],
                                ht[b][st][:, :, aoff : aoff + asz],
                                ht[b][st][:, :, :D1],
                                start=(st == 0),
                                stop=(st == NT8 - 1),
                                perf_mode=DR,
                            )
                    gsb = []
                    for ac, (aoff, asz) in enumerate(CH):
                        t = sq.tile([128, D1], bf16, tag=f"g{ac}", name=f"gs{b}_{ac}")
                        eng = nc.scalar.copy if ac % 2 == 0 else nc.vector.tensor_copy
                        eng(t[:asz, :], g_ps[ac][:asz, :])
                        gsb.append(t)

                    # ---- V = G P^T  (G symmetric: lhsT slices G directly)
                    v_ps = [
                        ppg.tile([128, D1], f32, tag=f"g{am}", name=f"v_ps{b}_{am}")
                        for am in range(3)
                    ]
                    for am, (amoff, amsz) in enumerate(CH):
                        for kb, (kboff, kbsz) in enumerate(CH):
                            nc.tensor.matmul(
                                v_ps[am][:amsz, :],
                                gsb[kb][:kbsz, amoff : amoff + amsz],
                                qpi[kb][:kbsz, D1 : 2 * D1],
                                start=(kb == 0),
                                stop=(kb == 2),
                            )
                    vsb = []
                    for am, (amoff, amsz) in enumerate(CH):
                        t = sq.tile([128, D1], bf16, tag=f"v{am}", name=f"vs{b}_{am}")
                        eng = nc.scalar.copy if am % 2 == 1 else nc.vector.tensor_copy
                        eng(t[:amsz, :], v_ps[am][:amsz, :])
                        vsb.append(t)

                    # ---- Et = I + (Q/n)^T V  (= I + C^T/n)
                    c_ps = [
                        ppg.tile([128, D1], f32, tag=f"g{em}", name=f"c_ps{b}_{em}")
                        for em in range(3)
                    ]
                    for em, (emoff, emsz) in enumerate(CH):
                        for ka, (kaoff, kasz) in enumerate(CH):
                            nc.tensor.matmul(
                                c_ps[em][:emsz, :],
                                qpi[ka][:kasz, emoff : emoff + emsz],
                                vsb[ka][:kasz, :],
                                start=(ka == 0),
                                stop=(ka == 2),
                            )
                    et = []
                    for em, (emoff, emsz) in enumerate(CH):
                        t = sq.tile([128, D1], bf16, tag=f"e{em}", name=f"et{b}_{em}")
                        nc.vector.tensor_add(
                            t[:emsz, :],
                            c_ps[em][:emsz, :],
                            qpi[em][:emsz, 2 * D1 : 3 * D1],
                        )
                        et.append(t)

                    # ---- Y = Et^T H
                    ei = 0
                    for dc, (doff, dsz) in enumerate(CH):
                        y = yp.tile([128, N1], bf16, tag=f"y{dc}", name=f"y{b}_{dc}")
                        for toff, tsz in TCH:
                            p = pp4.tile(
                                [128, 512], f32, tag="p", name=f"p4_{b}_{dc}_{toff}"
                            )
                            for ec, (eoff, esz) in enumerate(CH):
                                nc.tensor.matmul(
                                    p[:dsz, :tsz],
                                    et[ec][:esz, doff : doff + dsz],
                                    hb[b][ec][:esz, toff : toff + tsz],
                                    start=(ec == 0),
                                    stop=(ec == 2),
                                )
                            eng = (
                                nc.scalar.copy if ei % 2 == 0 else nc.vector.tensor_copy
                            )
                            eng(y[:dsz, toff : toff + tsz], p[:dsz, :tsz])
                            st_eng = nc.sync if ei % 2 == 0 else nc.gpsimd
                            st_eng.dma_start(
                                Y_d[b, doff : doff + dsz, toff : toff + tsz],
                                y[:dsz, toff : toff + tsz],
                            )
                            ei += 1

    nc.compile()
    return nc


def _prep_in_maps(H, P, Q):
    bf = ml_dtypes.bfloat16
    f8 = ml_dtypes.float8_e4m3
    H = np.ascontiguousarray(H, dtype=np.float32)
    Hb = H.astype(bf)
    # DoubleRow packing: [st, p, i, d] with s = st*256 + i*128 + p
    Ht = np.swapaxes(H[:, :, :N], 1, 2).reshape(B, NT8, 2, 128, D1)
    Ht8 = np.zeros((B, NT8, 128, 2, DPAD), dtype=f8)
    Ht8[..., :D1] = np.swapaxes(Ht, 2, 3).astype(f8)
    Ht = Ht8
    QPI = np.concatenate(
        [Q / N, P.T, np.eye(D1, dtype=np.float32)], axis=1
    ).astype(bf)
    QPI = np.ascontiguousarray(QPI)
    return [
        {
            "Hb": Hb[c * BPC : (c + 1) * BPC],
            "Ht": Ht[c * BPC : (c + 1) * BPC],
            "QPI": QPI,
        }
        for c in range(NCORES)
    ]


def kernel(H, P, Q):
    from concourse.bass_utils import run_bass_kernel_spmd

    if "nc" not in _cached:
        _cached["nc"] = _build()
    nc = _cached["nc"]

    in_maps = _prep_in_maps(H, P, Q)
    res = run_bass_kernel_spmd(nc, in_maps, list(range(NCORES)))
    out = np.concatenate([res.results[c]["Y"] for c in range(NCORES)], axis=0)
    return out.astype(np.float32)
